# revision 5
# baseline (speedup 1.0000x reference)
"""Expert-parallel MoE GroupedMLP kernel for 8 Trainium2 NeuronCores.

Problem: T=4096 tokens, H=2048 hidden, E=8 experts, I=4096 intermediate,
top_k=2, fp32 reference.

Strategy (expert-parallel, sharded inside kernel()):
  - Host: softmax + top-k routing, all-to-all token dispatch (gather each
    expert's assigned tokens), weight transpose + bf16 cast.  This is the
    sharding/unsharding step; all heavy FLOPs run on device.
  - Device (one expert per core): batched MLP over the expert's gathered
    tokens, padded to capacity C.  bf16 matmuls with fp32 PSUM
    accumulation; SiLU on the scalar engine; combine-weight scaling on the
    vector engine.
  - Host: scatter-add the 8 per-expert outputs into the full [T, H] result.
"""

import numpy as np
import ml_dtypes

from concourse import bass, bacc, tile, mybir
from concourse.bass_utils import run_bass_kernel_spmd

# Problem dims (hardcoded per contract)
T, H, E, I = 4096, 2048, 8, 4096
P = 128          # partitions
C = 1536         # per-expert token capacity (mean load is 1024, ~17 sigma margin)
TCH = 512        # token chunk = matmul moving dim / PSUM bank width (fp32)
NT = C // TCH    # 3 token chunks
KH = H // P      # 16 contraction tiles for MM1
NJ = I // P      # 32 intermediate tiles
HCH = 512        # output hidden chunk
NH = H // HCH    # 4
TT = TCH // P    # 4 token tiles per chunk

_BF16 = mybir.dt.bfloat16
_F32 = mybir.dt.float32


def build_kernel():
    nc = bacc.Bacc("TRN2", target_bir_lowering=False, debug=False, num_devices=E)
    xg_d = nc.dram_tensor("xg", [H, C], _BF16, kind="ExternalInput").ap()
    w1t_d = nc.dram_tensor("w1t", [H, 2 * I], _BF16, kind="ExternalInput").ap()
    w2t_d = nc.dram_tensor("w2t", [I, H], _BF16, kind="ExternalInput").ap()
    cg_d = nc.dram_tensor("cg", [C, 1], _F32, kind="ExternalInput").ap()
    yg_d = nc.dram_tensor("yg", [C, H], _F32, kind="ExternalOutput").ap()

    AF = mybir.ActivationFunctionType

    with tile.TileContext(nc) as tc:
        with (
            tc.tile_pool(name="xp", bufs=2) as xp,
            tc.tile_pool(name="w1p", bufs=2) as w1p,
            tc.tile_pool(name="w2p", bufs=2) as w2p,
            tc.tile_pool(name="actp", bufs=1) as actp,
            tc.tile_pool(name="cp", bufs=1) as cp,
            tc.tile_pool(name="sp", bufs=3) as sp,
            tc.tile_pool(name="op", bufs=3) as op,
            tc.tile_pool(name="ps1", bufs=2, space="PSUM") as ps1,
            tc.tile_pool(name="ps2", bufs=2, space="PSUM") as ps2,
        ):
            # combine weights, one [128,1] tile per token tile
            ctiles = []
            for q in range(NT * TT):
                ct = cp.tile([P, 1], _F32, tag=f"c{q}")
                nc.sync.dma_start(out=ct[:], in_=cg_d[q * P:(q + 1) * P, :])
                ctiles.append(ct)

            for tch in range(NT):
                # gathered tokens for this chunk: [128, 16 h-tiles, 512 tok]
                xt = xp.tile([P, KH, TCH], _BF16, tag="xg")
                nc.sync.dma_start(
                    out=xt[:],
                    in_=xg_d[:, tch * TCH:(tch + 1) * TCH].rearrange(
                        "(k p) t -> p k t", p=P),
                )
                # ---- phase A: h1 = x @ w1.T, act = silu(gate)*up ----
                acts = []
                for jp in range(NJ // 2):
                    n0 = jp * 2 * P
                    g = w1p.tile([P, KH, 2 * P], _BF16, tag="w1g")
                    u = w1p.tile([P, KH, 2 * P], _BF16, tag="w1u")
                    nc.sync.dma_start(
                        out=g[:],
                        in_=w1t_d[:, n0:n0 + 2 * P].rearrange(
                            "(k p) n -> p k n", p=P))
                    nc.sync.dma_start(
                        out=u[:],
                        in_=w1t_d[:, I + n0:I + n0 + 2 * P].rearrange(
                            "(k p) n -> p k n", p=P))
                    for lj in range(2):
                        j = jp * 2 + lj
                        pg = ps1.tile([P, TCH], _F32, tag="pg")
                        pu = ps1.tile([P, TCH], _F32, tag="pu")
                        for k in range(KH):
                            nc.tensor.matmul(
                                pg[:], g[:, k, lj * P:(lj + 1) * P],
                                xt[:, k, :], start=(k == 0), stop=(k == KH - 1))
                        for k in range(KH):
                            nc.tensor.matmul(
                                pu[:], u[:, k, lj * P:(lj + 1) * P],
                                xt[:, k, :], start=(k == 0), stop=(k == KH - 1))
                        st = sp.tile([P, TCH], _F32, tag="silu")
                        nc.scalar.activation(st[:], pg[:], AF.Sigmoid)
                        nc.vector.tensor_mul(st[:], st[:], pg[:])
                        at = actp.tile([P, TCH], _BF16, tag=f"act{j}")
                        nc.vector.tensor_mul(at[:], st[:], pu[:])
                        acts.append(at)
                # ---- phase B: y = act @ w2.T, scaled by combine weight ----
                for hc in range(NH):
                    w2s = w2p.tile([P, NJ, HCH], _BF16, tag="w2")
                    nc.sync.dma_start(
                        out=w2s[:],
                        in_=w2t_d[:, hc * HCH:(hc + 1) * HCH].rearrange(
                            "(j p) h -> p j h", p=P))
                    for tt in range(TT):
                        po = ps2.tile([P, HCH], _F32, tag="po")
                        for j in range(NJ):
                            nc.tensor.matmul(
                                po[:], acts[j][:, tt * P:(tt + 1) * P],
                                w2s[:, j, :], start=(j == 0), stop=(j == NJ - 1))
                        ot = op.tile([P, HCH], _F32, tag="out")
                        q = tch * TT + tt
                        nc.vector.tensor_scalar_mul(ot[:], po[:], ctiles[q][:])
                        nc.sync.dma_start(
                            out=yg_d[q * P:(q + 1) * P, hc * HCH:(hc + 1) * HCH],
                            in_=ot[:])
    nc.compile()
    return nc


_NC = None
LAST_RESULTS = []   # BassKernelResults of each wave of the last kernel() call


def _get_nc():
    global _NC
    if _NC is None:
        _NC = build_kernel()
    return _NC


def _route(router_logits, top_k):
    """Host routing: stable softmax + top-k (ties broken by lower index,
    matching jax.lax.top_k)."""
    logits = np.asarray(router_logits, dtype=np.float32)
    m = logits.max(axis=-1, keepdims=True)
    p = np.exp(logits - m)
    p /= p.sum(axis=-1, keepdims=True)
    ids = np.argsort(-p, axis=-1, kind="stable")[:, :top_k]   # [T, k]
    gates = np.take_along_axis(p, ids, axis=-1)               # [T, k]
    return ids, gates


def kernel(hidden_states, router_logits, w1, w2, top_k):
    top_k = int(top_k)
    x = np.asarray(hidden_states, dtype=np.float32)
    w1 = np.asarray(w1, dtype=np.float32)
    w2 = np.asarray(w2, dtype=np.float32)
    n_tok, hidden = x.shape
    n_exp = w1.shape[0]
    assert (n_tok, hidden, n_exp) == (T, H, E), "compiled for fixed shapes"

    ids, gates = _route(router_logits, top_k)

    # per-expert token lists
    expert_of = ids.ravel()
    token_of = np.repeat(np.arange(n_tok, dtype=np.int64), top_k)
    gate_of = gates.ravel()
    order = np.argsort(expert_of, kind="stable")
    expert_sorted = expert_of[order]
    token_sorted = token_of[order]
    gate_sorted = gate_of[order]
    counts = np.bincount(expert_sorted, minlength=n_exp)
    starts = np.concatenate([[0], np.cumsum(counts)])

    xT = x.T.astype(ml_dtypes.bfloat16)          # [H, T], contiguous
    w1t = [w1[e].T.astype(ml_dtypes.bfloat16) for e in range(n_exp)]
    w2t = [w2[e].T.astype(ml_dtypes.bfloat16) for e in range(n_exp)]

    nc = _get_nc()
    LAST_RESULTS.clear()
    out = np.zeros((n_tok, hidden), dtype=np.float32)
    done = np.zeros(n_exp, dtype=np.int64)   # tokens dispatched per expert
    while True:
        waves = []
        for e in range(n_exp):
            lo = starts[e] + done[e]
            hi = min(starts[e + 1], lo + C)
            waves.append((lo, hi))
        if all(lo >= hi for lo, hi in waves):
            break
        in_maps = []
        toks_per_e = []
        for e, (lo, hi) in enumerate(waves):
            n_e = hi - lo
            toks = token_sorted[lo:hi]
            toks_per_e.append(toks)
            xg = np.zeros((H, C), dtype=ml_dtypes.bfloat16)
            cg = np.zeros((C, 1), dtype=np.float32)
            if n_e:
                xg[:, :n_e] = xT[:, toks]
                cg[:n_e, 0] = gate_sorted[lo:hi]
            in_maps.append({"xg": xg, "w1t": w1t[e], "w2t": w2t[e], "cg": cg})
            done[e] += n_e
        res = run_bass_kernel_spmd(nc, in_maps, list(range(E)))
        LAST_RESULTS.append(res)
        for e in range(n_exp):
            toks = toks_per_e[e]
            if len(toks):
                out[toks] += res.results[e]["yg"][:len(toks)]
    return out


# revision 10
# speedup vs baseline: 1.1919x; 1.1919x over previous
"""Expert-parallel MoE GroupedMLP kernel for 8 Trainium2 NeuronCores.

Problem: T=4096 tokens, H=2048 hidden, E=8 experts, I=4096 intermediate,
top_k=2, fp32 reference.

Strategy (expert-parallel, sharded inside kernel()):
  - Host: softmax + top-k routing, all-to-all token dispatch (gather each
    expert's assigned tokens), weight transpose + bf16 cast.  This is the
    sharding/unsharding step; all heavy FLOPs run on device.
  - Device (one expert per core): batched MLP over the expert's gathered
    tokens, padded to capacity C.  bf16 matmuls with fp32 PSUM
    accumulation; SiLU on the scalar engine; combine-weight scaling on the
    vector engine.
  - Host: scatter-add the 8 per-expert outputs into the full [T, H] result.
"""

import numpy as np
import ml_dtypes

from concourse import bass, bacc, tile, mybir
from concourse.bass_utils import run_bass_kernel_spmd

# Problem dims (hardcoded per contract)
T, H, E, I = 4096, 2048, 8, 4096
P = 128          # partitions
KH = H // P      # 16 contraction tiles for MM1
NJ = I // P      # 32 intermediate tiles
HCH = 512        # output hidden chunk
NH = H // HCH    # 4

_BF16 = mybir.dt.bfloat16
_F32 = mybir.dt.float32


def chunk_plan(max_count):
    """Token chunk sizes covering the max per-expert load.  Chunks are the
    matmul moving dim: <=512 (one fp32 PSUM bank), multiples of 128."""
    cap = max(P, -(-max_count // P) * P)
    plan = [512] * (cap // 512)
    if cap % 512:
        plan.append(cap % 512)
    return tuple(plan)


def build_kernel(plan):
    C = sum(plan)
    nc = bacc.Bacc("TRN2", target_bir_lowering=False, debug=False, num_devices=E)
    xg_d = nc.dram_tensor("xg", [H, C], _BF16, kind="ExternalInput").ap()
    w1t_d = nc.dram_tensor("w1t", [H, 2 * I], _BF16, kind="ExternalInput").ap()
    w2t_d = nc.dram_tensor("w2t", [I, H], _BF16, kind="ExternalInput").ap()
    cg_d = nc.dram_tensor("cg", [C, 1], _F32, kind="ExternalInput").ap()
    yg_d = nc.dram_tensor("yg", [C, H], _F32, kind="ExternalOutput").ap()

    AF = mybir.ActivationFunctionType

    with tile.TileContext(nc) as tc:
        with (
            tc.tile_pool(name="xp", bufs=2) as xp,
            tc.tile_pool(name="w1p", bufs=2) as w1p,
            tc.tile_pool(name="w2p", bufs=2) as w2p,
            tc.tile_pool(name="actp", bufs=1) as actp,
            tc.tile_pool(name="cp", bufs=1) as cp,
            tc.tile_pool(name="sp", bufs=3) as sp,
            tc.tile_pool(name="op", bufs=3) as op,
            tc.tile_pool(name="ps1", bufs=2, space="PSUM") as ps1,
            tc.tile_pool(name="ps2", bufs=2, space="PSUM") as ps2,
        ):
            # combine weights, one [128,1] tile per token tile
            ctiles = []
            for q in range(C // P):
                ct = cp.tile([P, 1], _F32, tag=f"c{q}")
                nc.sync.dma_start(out=ct[:], in_=cg_d[q * P:(q + 1) * P, :])
                ctiles.append(ct)

            t_done = 0
            for tch, TCH in enumerate(plan):
                TT = TCH // P
                t0 = t_done
                t_done += TCH
                # gathered tokens for this chunk: [128, 16 h-tiles, TCH tok]
                xt = xp.tile([P, KH, TCH], _BF16, tag="xg")
                nc.sync.dma_start(
                    out=xt[:],
                    in_=xg_d[:, t0:t0 + TCH].rearrange(
                        "(k p) t -> p k t", p=P),
                )
                # ---- phase A: h1 = x @ w1.T, act = silu(gate)*up ----
                acts = []
                for jp in range(NJ // 2):
                    n0 = jp * 2 * P
                    g = w1p.tile([P, KH, 2 * P], _BF16, tag="w1g")
                    u = w1p.tile([P, KH, 2 * P], _BF16, tag="w1u")
                    nc.sync.dma_start(
                        out=g[:],
                        in_=w1t_d[:, n0:n0 + 2 * P].rearrange(
                            "(k p) n -> p k n", p=P))
                    nc.sync.dma_start(
                        out=u[:],
                        in_=w1t_d[:, I + n0:I + n0 + 2 * P].rearrange(
                            "(k p) n -> p k n", p=P))
                    for lj in range(2):
                        j = jp * 2 + lj
                        pg = ps1.tile([P, TCH], _F32, tag="pg")
                        pu = ps1.tile([P, TCH], _F32, tag="pu")
                        for k in range(KH):
                            nc.tensor.matmul(
                                pg[:], g[:, k, lj * P:(lj + 1) * P],
                                xt[:, k, :], start=(k == 0), stop=(k == KH - 1))
                        for k in range(KH):
                            nc.tensor.matmul(
                                pu[:], u[:, k, lj * P:(lj + 1) * P],
                                xt[:, k, :], start=(k == 0), stop=(k == KH - 1))
                        st = sp.tile([P, TCH], _F32, tag="silu")
                        nc.scalar.activation(st[:], pg[:], AF.Sigmoid)
                        nc.vector.tensor_mul(st[:], st[:], pg[:])
                        at = actp.tile([P, TCH], _BF16, tag=f"act{j}")
                        nc.vector.tensor_mul(at[:], st[:], pu[:])
                        acts.append(at)
                # ---- phase B: y = act @ w2.T, scaled by combine weight ----
                for hc in range(NH):
                    w2s = w2p.tile([P, NJ, HCH], _BF16, tag="w2")
                    nc.sync.dma_start(
                        out=w2s[:],
                        in_=w2t_d[:, hc * HCH:(hc + 1) * HCH].rearrange(
                            "(j p) h -> p j h", p=P))
                    for tt in range(TT):
                        po = ps2.tile([P, HCH], _F32, tag="po")
                        for j in range(NJ):
                            nc.tensor.matmul(
                                po[:], acts[j][:, tt * P:(tt + 1) * P],
                                w2s[:, j, :], start=(j == 0), stop=(j == NJ - 1))
                        ot = op.tile([P, HCH], _F32, tag="out")
                        q = t0 // P + tt
                        nc.vector.tensor_scalar_mul(ot[:], po[:], ctiles[q][:])
                        nc.sync.dma_start(
                            out=yg_d[q * P:(q + 1) * P, hc * HCH:(hc + 1) * HCH],
                            in_=ot[:])
    nc.compile()
    return nc


_NC_CACHE = {}
LAST_RESULTS = []   # BassKernelResults of each wave of the last kernel() call


def _get_nc(plan):
    if plan not in _NC_CACHE:
        _NC_CACHE[plan] = build_kernel(plan)
    return _NC_CACHE[plan]


def _route(router_logits, top_k):
    """Host routing: stable softmax + top-k (ties broken by lower index,
    matching jax.lax.top_k)."""
    logits = np.asarray(router_logits, dtype=np.float32)
    m = logits.max(axis=-1, keepdims=True)
    p = np.exp(logits - m)
    p /= p.sum(axis=-1, keepdims=True)
    ids = np.argsort(-p, axis=-1, kind="stable")[:, :top_k]   # [T, k]
    gates = np.take_along_axis(p, ids, axis=-1)               # [T, k]
    return ids, gates


def kernel(hidden_states, router_logits, w1, w2, top_k):
    top_k = int(top_k)
    x = np.asarray(hidden_states, dtype=np.float32)
    w1 = np.asarray(w1, dtype=np.float32)
    w2 = np.asarray(w2, dtype=np.float32)
    n_tok, hidden = x.shape
    n_exp = w1.shape[0]
    assert (n_tok, hidden, n_exp) == (T, H, E), "compiled for fixed shapes"

    ids, gates = _route(router_logits, top_k)

    # per-expert token lists
    expert_of = ids.ravel()
    token_of = np.repeat(np.arange(n_tok, dtype=np.int64), top_k)
    gate_of = gates.ravel()
    order = np.argsort(expert_of, kind="stable")
    expert_sorted = expert_of[order]
    token_sorted = token_of[order]
    gate_sorted = gate_of[order]
    counts = np.bincount(expert_sorted, minlength=n_exp)
    starts = np.concatenate([[0], np.cumsum(counts)])

    xT = x.T.astype(ml_dtypes.bfloat16)          # [H, T], contiguous
    w1t = [w1[e].T.astype(ml_dtypes.bfloat16) for e in range(n_exp)]
    w2t = [w2[e].T.astype(ml_dtypes.bfloat16) for e in range(n_exp)]

    plan = chunk_plan(int(counts.max()))
    C = sum(plan)
    nc = _get_nc(plan)
    LAST_RESULTS.clear()
    out = np.zeros((n_tok, hidden), dtype=np.float32)
    done = np.zeros(n_exp, dtype=np.int64)   # tokens dispatched per expert
    while True:
        waves = []
        for e in range(n_exp):
            lo = starts[e] + done[e]
            hi = min(starts[e + 1], lo + C)
            waves.append((lo, hi))
        if all(lo >= hi for lo, hi in waves):
            break
        in_maps = []
        toks_per_e = []
        for e, (lo, hi) in enumerate(waves):
            n_e = hi - lo
            toks = token_sorted[lo:hi]
            toks_per_e.append(toks)
            xg = np.zeros((H, C), dtype=ml_dtypes.bfloat16)
            cg = np.zeros((C, 1), dtype=np.float32)
            if n_e:
                xg[:, :n_e] = xT[:, toks]
                cg[:n_e, 0] = gate_sorted[lo:hi]
            in_maps.append({"xg": xg, "w1t": w1t[e], "w2t": w2t[e], "cg": cg})
            done[e] += n_e
        res = run_bass_kernel_spmd(nc, in_maps, list(range(E)))
        LAST_RESULTS.append(res)
        for e in range(n_exp):
            toks = toks_per_e[e]
            if len(toks):
                out[toks] += res.results[e]["yg"][:len(toks)]
    return out


# revision 13
# speedup vs baseline: 1.2949x; 1.0864x over previous
"""Expert-parallel MoE GroupedMLP kernel for 8 Trainium2 NeuronCores.

Problem: T=4096 tokens, H=2048 hidden, E=8 experts, I=4096 intermediate,
top_k=2, fp32 reference.

Strategy (expert-parallel, sharded inside kernel()):
  - Host: softmax + top-k routing, all-to-all token dispatch (gather each
    expert's assigned tokens), weight transpose + bf16 cast.  This is the
    sharding/unsharding step; all heavy FLOPs run on device.
  - Device (one expert per core): batched MLP over the expert's gathered
    tokens, padded to capacity C.  bf16 matmuls with fp32 PSUM
    accumulation; SiLU on the scalar engine; combine-weight scaling on the
    vector engine.
  - Host: scatter-add the 8 per-expert outputs into the full [T, H] result.
"""

import numpy as np
import ml_dtypes

from concourse import bass, bacc, tile, mybir
from concourse.bass_utils import run_bass_kernel_spmd

# Problem dims (hardcoded per contract)
T, H, E, I = 4096, 2048, 8, 4096
P = 128          # partitions
KH = H // P      # 16 contraction tiles for MM1
NJ = I // P      # 32 intermediate tiles
HCH = 512        # output hidden chunk
NH = H // HCH    # 4

_BF16 = mybir.dt.bfloat16
_F32 = mybir.dt.float32


def chunk_plan(max_count):
    """Token chunk sizes covering the max per-expert load.  Chunks are the
    matmul moving dim: <=512 (one fp32 PSUM bank), multiples of 128.  At
    most 3 chunks (2*3 MM1 psum banks + 2 MM2 banks = 8); larger loads are
    handled by multiple waves in kernel()."""
    cap = max(P, -(-max_count // P) * P)
    cap = min(cap, 1536)
    plan = [512] * (cap // 512)
    if cap % 512:
        plan.append(cap % 512)
    return tuple(plan)


def build_kernel(plan):
    C = sum(plan)
    nc = bacc.Bacc("TRN2", target_bir_lowering=False, debug=False, num_devices=E)
    xg_d = nc.dram_tensor("xg", [H, C], _BF16, kind="ExternalInput").ap()
    w1t_d = nc.dram_tensor("w1t", [H, 2 * I], _BF16, kind="ExternalInput").ap()
    w2t_d = nc.dram_tensor("w2t", [I, H], _BF16, kind="ExternalInput").ap()
    cg_d = nc.dram_tensor("cg", [C, 1], _F32, kind="ExternalInput").ap()
    yg_d = nc.dram_tensor("yg", [C, H], _F32, kind="ExternalOutput").ap()

    AF = mybir.ActivationFunctionType

    nchunks = len(plan)
    offs = [sum(plan[:i]) for i in range(nchunks)]
    JG = 4                    # w2 sub-slab j-group
    with tile.TileContext(nc) as tc:
        with (
            tc.tile_pool(name="xp", bufs=1) as xp,
            tc.tile_pool(name="w1p", bufs=2) as w1p,
            tc.tile_pool(name="w2p", bufs=NJ // JG + 1) as w2p,
            tc.tile_pool(name="actp", bufs=1) as actp,
            tc.tile_pool(name="cp", bufs=1) as cp,
            tc.tile_pool(name="sp", bufs=2) as sp,
            tc.tile_pool(name="op", bufs=3) as op,
            tc.tile_pool(name="psA", bufs=1, space="PSUM") as psA,
            tc.tile_pool(name="psB", bufs=2, space="PSUM") as psB,
        ):
            # combine weights, one [128,1] tile per token tile
            ctiles = []
            for q in range(C // P):
                ct = cp.tile([P, 1], _F32, tag=f"c{q}")
                nc.sync.dma_start(out=ct[:], in_=cg_d[q * P:(q + 1) * P, :])
                ctiles.append(ct)

            # gathered tokens, fully resident: one [128, C] tile per h-tile
            xtiles = []
            for k in range(KH):
                xk = xp.tile([P, C], _BF16, tag=f"x{k}")
                nc.sync.dma_start(out=xk[:], in_=xg_d[k * P:(k + 1) * P, :])
                xtiles.append(xk)

            # ---- phase A: h1 = x @ w1.T, act = silu(gate)*up ----
            # j-outer: w1 streamed exactly once; all chunks per PSUM group.
            acts = []
            for jp in range(NJ // 2):
                n0 = jp * 2 * P
                g = w1p.tile([P, KH, 2 * P], _BF16, tag="w1g")
                u = w1p.tile([P, KH, 2 * P], _BF16, tag="w1u")
                nc.sync.dma_start(
                    out=g[:],
                    in_=w1t_d[:, n0:n0 + 2 * P].rearrange(
                        "(k p) n -> p k n", p=P))
                nc.sync.dma_start(
                    out=u[:],
                    in_=w1t_d[:, I + n0:I + n0 + 2 * P].rearrange(
                        "(k p) n -> p k n", p=P))
                for lj in range(2):
                    j = jp * 2 + lj
                    lsl = slice(lj * P, (lj + 1) * P)
                    pgs = [psA.tile([P, pl], _F32, tag=f"pg{c}",
                                    name=f"pg{c}_{j}")
                           for c, pl in enumerate(plan)]
                    pus = [psA.tile([P, pl], _F32, tag=f"pu{c}",
                                    name=f"pu{c}_{j}")
                           for c, pl in enumerate(plan)]
                    for k in range(KH):
                        for c, pl in enumerate(plan):
                            nc.tensor.matmul(
                                pgs[c][:], g[:, k, lsl],
                                xtiles[k][:, offs[c]:offs[c] + pl],
                                start=(k == 0), stop=(k == KH - 1))
                    for k in range(KH):
                        for c, pl in enumerate(plan):
                            nc.tensor.matmul(
                                pus[c][:], u[:, k, lsl],
                                xtiles[k][:, offs[c]:offs[c] + pl],
                                start=(k == 0), stop=(k == KH - 1))
                    at = actp.tile([P, C], _BF16, tag=f"act{j}")
                    for c, pl in enumerate(plan):
                        st = sp.tile([P, pl], _F32, tag="silu")
                        nc.scalar.activation(st[:], pgs[c][:], AF.Sigmoid)
                        nc.vector.tensor_mul(st[:], st[:], pgs[c][:])
                        nc.vector.tensor_mul(
                            at[:, offs[c]:offs[c] + pl], st[:], pus[c][:])
                    acts.append(at)

            # ---- phase B: y = act @ w2.T, scaled by combine weight ----
            for hc in range(NH):
                w2subs = []
                for jg in range(NJ // JG):
                    w2s = w2p.tile([P, JG, HCH], _BF16, tag="w2")
                    nc.sync.dma_start(
                        out=w2s[:],
                        in_=w2t_d[jg * JG * P:(jg + 1) * JG * P,
                                  hc * HCH:(hc + 1) * HCH].rearrange(
                            "(j p) h -> p j h", p=P))
                    w2subs.append(w2s)
                for tq in range(C // P):
                    po = psB.tile([P, HCH], _F32, tag="po")
                    for j in range(NJ):
                        nc.tensor.matmul(
                            po[:], acts[j][:, tq * P:(tq + 1) * P],
                            w2subs[j // JG][:, j % JG, :],
                            start=(j == 0), stop=(j == NJ - 1))
                    ot = op.tile([P, HCH], _F32, tag="out")
                    nc.vector.tensor_scalar_mul(ot[:], po[:], ctiles[tq][:])
                    nc.sync.dma_start(
                        out=yg_d[tq * P:(tq + 1) * P, hc * HCH:(hc + 1) * HCH],
                        in_=ot[:])
    nc.compile()
    return nc


_NC_CACHE = {}
LAST_RESULTS = []   # BassKernelResults of each wave of the last kernel() call


def _get_nc(plan):
    if plan not in _NC_CACHE:
        _NC_CACHE[plan] = build_kernel(plan)
    return _NC_CACHE[plan]


def _route(router_logits, top_k):
    """Host routing: stable softmax + top-k (ties broken by lower index,
    matching jax.lax.top_k)."""
    logits = np.asarray(router_logits, dtype=np.float32)
    m = logits.max(axis=-1, keepdims=True)
    p = np.exp(logits - m)
    p /= p.sum(axis=-1, keepdims=True)
    ids = np.argsort(-p, axis=-1, kind="stable")[:, :top_k]   # [T, k]
    gates = np.take_along_axis(p, ids, axis=-1)               # [T, k]
    return ids, gates


def kernel(hidden_states, router_logits, w1, w2, top_k):
    top_k = int(top_k)
    x = np.asarray(hidden_states, dtype=np.float32)
    w1 = np.asarray(w1, dtype=np.float32)
    w2 = np.asarray(w2, dtype=np.float32)
    n_tok, hidden = x.shape
    n_exp = w1.shape[0]
    assert (n_tok, hidden, n_exp) == (T, H, E), "compiled for fixed shapes"

    ids, gates = _route(router_logits, top_k)

    # per-expert token lists
    expert_of = ids.ravel()
    token_of = np.repeat(np.arange(n_tok, dtype=np.int64), top_k)
    gate_of = gates.ravel()
    order = np.argsort(expert_of, kind="stable")
    expert_sorted = expert_of[order]
    token_sorted = token_of[order]
    gate_sorted = gate_of[order]
    counts = np.bincount(expert_sorted, minlength=n_exp)
    starts = np.concatenate([[0], np.cumsum(counts)])

    xT = x.T.astype(ml_dtypes.bfloat16)          # [H, T], contiguous
    w1t = [w1[e].T.astype(ml_dtypes.bfloat16) for e in range(n_exp)]
    w2t = [w2[e].T.astype(ml_dtypes.bfloat16) for e in range(n_exp)]

    plan = chunk_plan(int(counts.max()))
    C = sum(plan)
    nc = _get_nc(plan)
    LAST_RESULTS.clear()
    out = np.zeros((n_tok, hidden), dtype=np.float32)
    done = np.zeros(n_exp, dtype=np.int64)   # tokens dispatched per expert
    while True:
        waves = []
        for e in range(n_exp):
            lo = starts[e] + done[e]
            hi = min(starts[e + 1], lo + C)
            waves.append((lo, hi))
        if all(lo >= hi for lo, hi in waves):
            break
        in_maps = []
        toks_per_e = []
        for e, (lo, hi) in enumerate(waves):
            n_e = hi - lo
            toks = token_sorted[lo:hi]
            toks_per_e.append(toks)
            xg = np.zeros((H, C), dtype=ml_dtypes.bfloat16)
            cg = np.zeros((C, 1), dtype=np.float32)
            if n_e:
                xg[:, :n_e] = xT[:, toks]
                cg[:n_e, 0] = gate_sorted[lo:hi]
            in_maps.append({"xg": xg, "w1t": w1t[e], "w2t": w2t[e], "cg": cg})
            done[e] += n_e
        res = run_bass_kernel_spmd(nc, in_maps, list(range(E)))
        LAST_RESULTS.append(res)
        for e in range(n_exp):
            toks = toks_per_e[e]
            if len(toks):
                out[toks] += res.results[e]["yg"][:len(toks)]
    return out


# revision 18
# speedup vs baseline: 1.3039x; 1.0070x over previous
"""Expert-parallel MoE GroupedMLP kernel for 8 Trainium2 NeuronCores.

Problem: T=4096 tokens, H=2048 hidden, E=8 experts, I=4096 intermediate,
top_k=2, fp32 reference.

Strategy (expert-parallel, sharded inside kernel()):
  - Host: softmax + top-k routing, all-to-all token dispatch (gather each
    expert's assigned tokens), weight transpose + bf16 cast.  This is the
    sharding/unsharding step; all heavy FLOPs run on device.
  - Device (one expert per core): batched MLP over the expert's gathered
    tokens, padded to capacity C.  bf16 matmuls with fp32 PSUM
    accumulation; SiLU on the scalar engine; combine-weight scaling on the
    vector engine.
  - Host: scatter-add the 8 per-expert outputs into the full [T, H] result.
"""

import numpy as np
import ml_dtypes

from concourse import bass, bacc, tile, mybir
from concourse.bass_utils import run_bass_kernel_spmd

# Problem dims (hardcoded per contract)
T, H, E, I = 4096, 2048, 8, 4096
P = 128          # partitions
KH = H // P      # 16 contraction tiles for MM1
NJ = I // P      # 32 intermediate tiles
HCH = 512        # output hidden chunk
NH = H // HCH    # 4

_BF16 = mybir.dt.bfloat16
_F32 = mybir.dt.float32


def chunk_plan(max_count):
    """Token chunk sizes covering the max per-expert load.  Chunks are the
    matmul moving dim: <=512 (one fp32 PSUM bank), multiples of 128.  At
    most 3 chunks (2*3 MM1 psum banks + 2 MM2 banks = 8); larger loads are
    handled by multiple waves in kernel()."""
    cap = max(P, -(-max_count // P) * P)
    cap = min(cap, 1536)
    plan = [512] * (cap // 512)
    if cap % 512:
        plan.append(cap % 512)
    return tuple(plan)


def build_kernel(plan):
    C = sum(plan)
    nc = bacc.Bacc("TRN2", target_bir_lowering=False, debug=False, num_devices=E)
    xg_d = nc.dram_tensor("xg", [H, C], _BF16, kind="ExternalInput").ap()
    w1t_d = nc.dram_tensor("w1t", [H, 2 * I], _BF16, kind="ExternalInput").ap()
    w2t_d = nc.dram_tensor("w2t", [I, H], _BF16, kind="ExternalInput").ap()
    # combine weights pre-tiled on host: cg[p, q] = weight of token q*128+p
    cg_d = nc.dram_tensor("cg", [P, C // P], _F32, kind="ExternalInput").ap()
    yg_d = nc.dram_tensor("yg", [C, H], _F32, kind="ExternalOutput").ap()

    AF = mybir.ActivationFunctionType

    nchunks = len(plan)
    offs = [sum(plan[:i]) for i in range(nchunks)]
    JG = 2                    # w2 sub-slab j-group
    with tile.TileContext(nc) as tc:
        with (
            tc.tile_pool(name="xp", bufs=1) as xp,
            tc.tile_pool(name="w1p", bufs=2) as w1p,
            tc.tile_pool(name="w2p", bufs=NJ // JG + 2) as w2p,
            tc.tile_pool(name="actp", bufs=1) as actp,
            tc.tile_pool(name="cp", bufs=1) as cp,
            tc.tile_pool(name="sp", bufs=2) as sp,
            tc.tile_pool(name="op", bufs=3) as op,
            tc.tile_pool(name="psA", bufs=1, space="PSUM") as psA,
            tc.tile_pool(name="psB", bufs=2, space="PSUM") as psB,
        ):
            # gathered tokens, fully resident: one [128, C] tile per h-tile
            xtiles = []
            for k in range(KH):
                xk = xp.tile([P, C], _BF16, tag=f"x{k}")
                nc.sync.dma_start(out=xk[:], in_=xg_d[k * P:(k + 1) * P, :])
                xtiles.append(xk)

            # ---- phase A: h1 = x @ w1.T, act = silu(gate)*up ----
            # j-outer: w1 streamed exactly once; all chunks per PSUM group.
            acts = []
            for jp in range(NJ // 2):
                n0 = jp * 2 * P
                g = w1p.tile([P, KH, 2 * P], _BF16, tag="w1g")
                u = w1p.tile([P, KH, 2 * P], _BF16, tag="w1u")
                nc.sync.dma_start(
                    out=g[:],
                    in_=w1t_d[:, n0:n0 + 2 * P].rearrange(
                        "(k p) n -> p k n", p=P))
                nc.sync.dma_start(
                    out=u[:],
                    in_=w1t_d[:, I + n0:I + n0 + 2 * P].rearrange(
                        "(k p) n -> p k n", p=P))
                for lj in range(2):
                    j = jp * 2 + lj
                    lsl = slice(lj * P, (lj + 1) * P)
                    pgs = [psA.tile([P, pl], _F32, tag=f"pg{c}",
                                    name=f"pg{c}_{j}")
                           for c, pl in enumerate(plan)]
                    pus = [psA.tile([P, pl], _F32, tag=f"pu{c}",
                                    name=f"pu{c}_{j}")
                           for c, pl in enumerate(plan)]
                    for k in range(KH):
                        for c, pl in enumerate(plan):
                            nc.tensor.matmul(
                                pgs[c][:], g[:, k, lsl],
                                xtiles[k][:, offs[c]:offs[c] + pl],
                                start=(k == 0), stop=(k == KH - 1))
                    for k in range(KH):
                        for c, pl in enumerate(plan):
                            nc.tensor.matmul(
                                pus[c][:], u[:, k, lsl],
                                xtiles[k][:, offs[c]:offs[c] + pl],
                                start=(k == 0), stop=(k == KH - 1))
                    at = actp.tile([P, C], _BF16, tag=f"act{j}")
                    for c, pl in enumerate(plan):
                        st = sp.tile([P, pl], _F32, tag="silu")
                        nc.scalar.activation(st[:], pgs[c][:], AF.Sigmoid)
                        nc.vector.tensor_mul(st[:], st[:], pgs[c][:])
                        nc.vector.tensor_mul(
                            at[:, offs[c]:offs[c] + pl], st[:], pus[c][:])
                    acts.append(at)

            # ---- phase B: y = act @ w2.T, scaled by combine weight ----
            ct = cp.tile([P, C // P], _F32, tag="cg")
            nc.sync.dma_start(out=ct[:], in_=cg_d[:])
            for hc in range(NH):
                w2subs = []
                for jg in range(NJ // JG):
                    w2s = w2p.tile([P, JG, HCH], _BF16, tag="w2")
                    nc.sync.dma_start(
                        out=w2s[:],
                        in_=w2t_d[jg * JG * P:(jg + 1) * JG * P,
                                  hc * HCH:(hc + 1) * HCH].rearrange(
                            "(j p) h -> p j h", p=P))
                    w2subs.append(w2s)
                for tq in range(C // P):
                    po = psB.tile([P, HCH], _F32, tag="po")
                    for j in range(NJ):
                        nc.tensor.matmul(
                            po[:], acts[j][:, tq * P:(tq + 1) * P],
                            w2subs[j // JG][:, j % JG, :],
                            start=(j == 0), stop=(j == NJ - 1))
                    ot = op.tile([P, HCH], _F32, tag="out")
                    nc.vector.tensor_scalar_mul(ot[:], po[:], ct[:, tq:tq + 1])
                    nc.sync.dma_start(
                        out=yg_d[tq * P:(tq + 1) * P, hc * HCH:(hc + 1) * HCH],
                        in_=ot[:])
    nc.compile()
    return nc


_NC_CACHE = {}
LAST_RESULTS = []   # BassKernelResults of each wave of the last kernel() call


def _get_nc(plan):
    if plan not in _NC_CACHE:
        _NC_CACHE[plan] = build_kernel(plan)
    return _NC_CACHE[plan]


def _route(router_logits, top_k):
    """Host routing: stable softmax + top-k (ties broken by lower index,
    matching jax.lax.top_k)."""
    logits = np.asarray(router_logits, dtype=np.float32)
    m = logits.max(axis=-1, keepdims=True)
    p = np.exp(logits - m)
    p /= p.sum(axis=-1, keepdims=True)
    ids = np.argsort(-p, axis=-1, kind="stable")[:, :top_k]   # [T, k]
    gates = np.take_along_axis(p, ids, axis=-1)               # [T, k]
    return ids, gates


def kernel(hidden_states, router_logits, w1, w2, top_k):
    top_k = int(top_k)
    x = np.asarray(hidden_states, dtype=np.float32)
    w1 = np.asarray(w1, dtype=np.float32)
    w2 = np.asarray(w2, dtype=np.float32)
    n_tok, hidden = x.shape
    n_exp = w1.shape[0]
    assert (n_tok, hidden, n_exp) == (T, H, E), "compiled for fixed shapes"

    ids, gates = _route(router_logits, top_k)

    # per-expert token lists
    expert_of = ids.ravel()
    token_of = np.repeat(np.arange(n_tok, dtype=np.int64), top_k)
    gate_of = gates.ravel()
    order = np.argsort(expert_of, kind="stable")
    expert_sorted = expert_of[order]
    token_sorted = token_of[order]
    gate_sorted = gate_of[order]
    counts = np.bincount(expert_sorted, minlength=n_exp)
    starts = np.concatenate([[0], np.cumsum(counts)])

    xT = x.T.astype(ml_dtypes.bfloat16)          # [H, T], contiguous
    w1t = [w1[e].T.astype(ml_dtypes.bfloat16) for e in range(n_exp)]
    w2t = [w2[e].T.astype(ml_dtypes.bfloat16) for e in range(n_exp)]

    plan = chunk_plan(int(counts.max()))
    C = sum(plan)
    nc = _get_nc(plan)
    LAST_RESULTS.clear()
    out = np.zeros((n_tok, hidden), dtype=np.float32)
    done = np.zeros(n_exp, dtype=np.int64)   # tokens dispatched per expert
    while True:
        waves = []
        for e in range(n_exp):
            lo = starts[e] + done[e]
            hi = min(starts[e + 1], lo + C)
            waves.append((lo, hi))
        if all(lo >= hi for lo, hi in waves):
            break
        in_maps = []
        toks_per_e = []
        for e, (lo, hi) in enumerate(waves):
            n_e = hi - lo
            toks = token_sorted[lo:hi]
            toks_per_e.append(toks)
            xg = np.zeros((H, C), dtype=ml_dtypes.bfloat16)
            cg = np.zeros((C,), dtype=np.float32)
            if n_e:
                xg[:, :n_e] = xT[:, toks]
                cg[:n_e] = gate_sorted[lo:hi]
            # pre-tile: cg_t[p, q] = cg[q*128 + p]
            cg = np.ascontiguousarray(cg.reshape(C // P, P).T)
            in_maps.append({"xg": xg, "w1t": w1t[e], "w2t": w2t[e], "cg": cg})
            done[e] += n_e
        res = run_bass_kernel_spmd(nc, in_maps, list(range(E)))
        LAST_RESULTS.append(res)
        for e in range(n_exp):
            toks = toks_per_e[e]
            if len(toks):
                out[toks] += res.results[e]["yg"][:len(toks)]
    return out
